# revision 16
# baseline (speedup 1.0000x reference)
"""Trainium2 Bass kernel for nn_BaseGCN (20-layer GraphConv, norm='both').

Distribution (graph/data parallel per the sharding hint): nodes are sharded
contiguously across 8 NeuronCores (12500 each, padded to 12544 = 98 tiles of
128), sorted by in-degree so each 128-node tile has near-uniform in-edge
count.  Per layer each core computes s = dout*h on its shard, AllGathers the
8 shards into a compact DRAM table, expands it to 256B-strided rows, then
fetches the source rows of its in-edges with batched `dma_gather`
instructions (thousands of rows per instruction).  The int16 gather index
forces a 4-way bank split of the 100352-row table; edge slots are laid out
per (bank, tile) so strided DVE reduces segment-sum each bank, the four
bank partials are added, W is applied via PE transpose + block-diagonal
matmul, then din / bias / relu.

Graph *structure* (shard assignment, sort order, slot layout, gather
indices, integer degrees) is computed on the host, like CSR construction in
any GNN framework.  All floating-point math runs on device.
"""

import numpy as np

P = 128   # SBUF partitions
RW = 64   # f32 per table row (256B, dma_gather granularity)
CC = 96   # max slot columns per dma_gather chunk


class _Cfg:
    def __init__(self, N, E, D, n_mid, n_cores):
        self.N = N
        self.E = E
        self.D = D
        self.n_mid = n_mid
        self.L = n_mid + 2
        self.NC = n_cores
        self.Np = N // n_cores          # 12500
        self.n_tiles = -(-self.Np // P)  # 98
        self.SH = self.n_tiles * P       # 12544 padded shard rows
        self.full_t = self.Np // P       # 97
        self.rem = self.Np - self.full_t * P  # 84
        self.NB = 4                      # index banks (int16 limit)
        self.BR = 2 * self.SH            # rows per bank = 25088


CFG = _Cfg(N=100000, E=3200000, D=20, n_mid=18, n_cores=8)


# ---------------------------------------------------------------------------
# Host-side planning (graph structure only)
# ---------------------------------------------------------------------------

def _plan(src, dst, cfg):
    N, NC, Np, n_tiles, SH, NB = (cfg.N, cfg.NC, cfg.Np, cfg.n_tiles, cfg.SH,
                                  cfg.NB)
    src = np.asarray(src, dtype=np.int64)
    dst = np.asarray(dst, dtype=np.int64)

    out_deg = np.bincount(src, minlength=N) + 1  # + self loop
    in_deg = np.bincount(dst, minlength=N) + 1

    core_of = np.arange(N) // Np
    # per-node per-source-bank in-edge counts; bank of a source node is its
    # core pair (gidx // (2*SH) == core // 2, independent of within-core pos)
    bank_of = core_of // 2
    cnb = np.zeros((N, NB), np.int32)
    np.add.at(cnb, (dst, bank_of[src]), 1)
    pos = np.empty(N, np.int64)
    perms = np.zeros((NC, Np), np.int64)
    for c in range(NC):
        nodes = np.arange(c * Np, (c + 1) * Np)
        # group nodes by dominant source bank, then by max / 2nd-max per-bank
        # count: minimizes per-tile per-bank slot maxima (34% fewer gather
        # rows than total-degree sort)
        cb = cnb[nodes]
        sc = np.sort(cb, axis=1)
        order = np.lexsort((sc[:, 2], sc[:, 3], cb.argmax(axis=1)))
        perms[c] = nodes[order]
        pos[nodes[order]] = np.arange(Np)
    gidx = core_of * SH + pos           # row in the expanded table
    bank = gidx // cfg.BR               # 4 banks of 25088 rows

    # per (core, tile, partition, bank) in-edge counts (self-loops local)
    cnt = np.zeros((NC, n_tiles, P, NB), np.int32)
    dp = pos[dst]
    np.add.at(cnt, (core_of[dst], dp // P, dp % P, bank[src]), 1)
    S = cnt.max(axis=2).max(axis=0)     # [n_tiles, NB] shared across cores

    off = np.zeros((NB, n_tiles + 1), np.int64)   # column offsets per bank
    for b in range(NB):
        off[b, 1:] = np.cumsum(S[:, b])
    REG = off[:, -1]                    # region sizes
    BOFF = np.concatenate([[0], np.cumsum(REG)])
    BTOT = int(BOFF[-1])

    # chunks: per bank, tile runs with <= CC columns
    chunks = []   # (bank, t0, t1, col0_rel, cols)
    for b in range(NB):
        t0 = 0
        while t0 < n_tiles:
            while t0 < n_tiles and S[t0, b] == 0:
                t0 += 1
            if t0 >= n_tiles:
                break
            t1 = t0 + 1
            while t1 < n_tiles and off[b, t1 + 1] - off[b, t0] <= CC:
                t1 += 1
            chunks.append((b, t0, t1, int(off[b, t0]),
                           int(off[b, t1] - off[b, t0])))
            t0 = t1

    # reduce jobs: per chunk, runs of equal S
    jobs = []   # (chunk_id, col_in_chunk, t0, T, S)
    for ci, (b, t0, t1, col0, _) in enumerate(chunks):
        t = t0
        while t < t1:
            te = t + 1
            while te < t1 and S[te, b] == S[t, b]:
                te += 1
            jobs.append((ci, int(off[b, t] - col0), int(t), int(te - t),
                         int(S[t, b])))
            t = te

    # gather indices: per core [P, BTOT] local bank row ids
    ZPAD = {b: 2 * b * SH + Np - cfg.BR * b for b in range(NB)}  # pad row
    idx = np.zeros((NC, P, BTOT), np.int32)
    for b in range(NB):
        idx[:, :, BOFF[b]:BOFF[b + 1]] = ZPAD[b]
    ekey = gidx[dst]
    eorder = np.argsort(ekey, kind="stable")
    s_sorted = src[eorder]
    d_sorted = dst[eorder]
    e_core = core_of[d_sorted]
    core_starts = np.searchsorted(e_core, np.arange(NC + 1))
    for c in range(NC):
        lo, hi = core_starts[c], core_starts[c + 1]
        ss = s_sorted[lo:hi]
        dpc = pos[d_sorted[lo:hi]]
        sb = bank[ss]
        # rank of each edge within its (dst-node, bank) group
        key = dpc * NB + sb
        korder = np.argsort(key, kind="stable")
        ksorted = key[korder]
        grp_cnt = np.bincount(ksorted, minlength=Np * NB)
        starts = np.concatenate([[0], np.cumsum(grp_cnt)[:-1]])
        rank = np.arange(len(ss)) - starts[ksorted]
        cols = (BOFF[sb[korder]] + off[sb[korder], dpc[korder] // P]
                + rank)
        lidx = gidx[ss[korder]] - sb[korder] * cfg.BR
        idx[c, dpc[korder] % P, cols] = lidx.astype(np.int32)

    # int16 wrapped layout: flat i = col*128 + p -> (p%16, col*8 + p//16)
    pp = np.arange(P)
    idx16 = np.zeros((NC, 16, BTOT * 8), np.int16)
    for c in range(NC):
        a = np.zeros((16, BTOT * 8), np.int16)
        a[(pp % 16)[:, None],
          np.add.outer(pp // 16, np.arange(BTOT) * 8)] = idx[c].astype(np.int16)
        idx16[c] = a

    def tiles_of(vec_by_core):
        out = np.zeros((NC, P, n_tiles), np.float32)
        for c in range(NC):
            v = np.zeros(SH, np.float32)
            v[:Np] = vec_by_core[c]
            out[c] = v.reshape(n_tiles, P).T
        return out

    indeg_t = tiles_of([np.maximum(in_deg[perms[c]], 1) for c in range(NC)])
    outdeg_t = tiles_of([np.maximum(out_deg[perms[c]], 1) for c in range(NC)])
    indeg_t[indeg_t == 0] = 1.0
    # pad nodes: dout = 1/sqrt(1e30) ~ 0 so their table rows stay ~0
    outdeg_t[outdeg_t == 0] = 1e30

    groups = [(g0, min(4, n_tiles - g0)) for g0 in range(0, n_tiles, 4)]
    group_sizes = sorted(set(gt for _, gt in groups))

    return dict(perms=perms, S=S, off=off, REG=REG, BOFF=BOFF, BTOT=BTOT,
                chunks=chunks, jobs=jobs, idx16=idx16, indeg_t=indeg_t,
                outdeg_t=outdeg_t, groups=groups, group_sizes=group_sizes)


def _pack_weights(W_start, b_start, W_mid, b_mid, W_final, b_final, plan, cfg):
    D, L = cfg.D, cfg.L
    Ws = [np.asarray(W_start, np.float32)]
    bs = [np.asarray(b_start, np.float32)]
    for i in range(cfg.n_mid):
        Ws.append(np.asarray(W_mid[i], np.float32))
        bs.append(np.asarray(b_mid[i], np.float32))
    Ws.append(np.asarray(W_final, np.float32))
    bs.append(np.asarray(b_final, np.float32))

    wstart_rep = np.tile(Ws[0][0][None, :], (P, 1)).astype(np.float32)
    brep = np.zeros((P, L * D), np.float32)
    for l in range(L):
        brep[:, l * D:(l + 1) * D] = bs[l][None, :]

    wblk = {}
    for gt in plan["group_sizes"]:
        M = np.zeros((gt * D, (L - 1) * gt * D), np.float32)
        for l in range(2, L + 1):
            W = Ws[l - 1]
            base = (l - 2) * gt * D
            for k in range(gt):
                M[k * D:(k + 1) * D, base + k * D:base + (k + 1) * D] = W
        wblk[gt] = M
    return wstart_rep, brep, wblk


# ---------------------------------------------------------------------------
# Device program
# ---------------------------------------------------------------------------

def build_program(plan, cfg):
    import concourse.bacc as bacc
    import concourse.mybir as mybir
    import concourse.tile as tile

    f32 = mybir.dt.float32
    i16 = mybir.dt.int16
    Alu = mybir.AluOpType
    Axis = mybir.AxisListType
    Act = mybir.ActivationFunctionType

    D, L, NC, Np = cfg.D, cfg.L, cfg.NC, cfg.Np
    n_tiles, full_t, rem, SH, NB, BR = (cfg.n_tiles, cfg.full_t, cfg.rem,
                                        cfg.SH, cfg.NB, cfg.BR)
    BTOT = plan["BTOT"]
    REG, BOFF = plan["REG"], plan["BOFF"]
    chunks, jobs = plan["chunks"], plan["jobs"]
    REGMAX = int(max(REG))
    TBL = NC * SH                        # 100352

    nc = bacc.Bacc("TRN2", target_bir_lowering=False, debug=False,
                   enable_asserts=False, num_devices=NC, num_swdge_queues=4)

    t_feat = nc.dram_tensor("feat_t", [P, n_tiles], f32, kind="ExternalInput").ap()
    t_indeg = nc.dram_tensor("indeg_t", [P, n_tiles], f32, kind="ExternalInput").ap()
    t_outdeg = nc.dram_tensor("outdeg_t", [P, n_tiles], f32, kind="ExternalInput").ap()
    t_idx16 = nc.dram_tensor("idx16", [16, BTOT * 8], i16, kind="ExternalInput").ap()
    t_wstart = nc.dram_tensor("wstart_rep", [P, D], f32, kind="ExternalInput").ap()
    t_brep = nc.dram_tensor("brep", [P, L * D], f32, kind="ExternalInput").ap()
    t_wblk = {}
    for gt in plan["group_sizes"]:
        t_wblk[gt] = nc.dram_tensor(f"wblk{gt}", [gt * D, (L - 1) * gt * D],
                                    f32, kind="ExternalInput").ap()
    t_ident = nc.dram_tensor("ident", [P, P], f32, kind="ExternalInput").ap()
    t_out = nc.dram_tensor("out_h", [Np, D], f32, kind="ExternalOutput").ap()

    rg = [list(range(NC))]

    with tile.TileContext(nc) as tc:
        with (
            tc.tile_pool(name="const", bufs=1) as const,
            tc.tile_pool(name="idxp", bufs=2) as idxp,
            tc.tile_pool(name="stream", bufs=2) as streamp,
            tc.tile_pool(name="aggp", bufs=1) as aggp,
            tc.tile_pool(name="stage", bufs=2) as stagep,
            tc.tile_pool(name="cbufp", bufs=2) as cbufp,
            tc.tile_pool(name="ebufp", bufs=1) as ebufp,
            tc.tile_pool(name="atp", bufs=2) as atp,
            tc.tile_pool(name="ptp", bufs=2, space="PSUM") as ptp,
            tc.tile_pool(name="php", bufs=2, space="PSUM") as php,
            tc.tile_pool(name="dram", bufs=1, space="DRAM") as dramp,
            tc.tile_pool(name="dramT", bufs=2, space="DRAM") as dramTp,
        ):
            # ---- prologue ----
            sb_feat = const.tile([P, n_tiles], f32, name="sb_feat")
            nc.sync.dma_start(out=sb_feat[:], in_=t_feat[:])
            sb_indeg = const.tile([P, n_tiles], f32, name="sb_indeg")
            nc.sync.dma_start(out=sb_indeg[:], in_=t_indeg[:])
            sb_outdeg = const.tile([P, n_tiles], f32, name="sb_outdeg")
            nc.sync.dma_start(out=sb_outdeg[:], in_=t_outdeg[:])
            sb_wstart = const.tile([P, D], f32, name="sb_wstart")
            nc.sync.dma_start(out=sb_wstart[:], in_=t_wstart[:])
            sb_brep = const.tile([P, L * D], f32, name="sb_brep")
            nc.sync.dma_start(out=sb_brep[:], in_=t_brep[:])
            sb_wblk = {}
            for gt in plan["group_sizes"]:
                sb_wblk[gt] = const.tile([gt * D, (L - 1) * gt * D], f32,
                                         name=f"sb_wblk{gt}")
                nc.sync.dma_start(out=sb_wblk[gt][:], in_=t_wblk[gt][:])
            sb_ident = const.tile([P, P], f32, name="sb_ident")
            nc.sync.dma_start(out=sb_ident[:], in_=t_ident[:])

            # replicate the 16-row idx block to 128 partitions in DRAM
            d_idx = dramp.tile([P, BTOT * 8], i16, name="d_idx", tag="d_idx")
            for g in range(8):
                nc.sync.dma_start(out=d_idx[16 * g:16 * (g + 1), :],
                                  in_=t_idx16[:, :])

            sb_tmp = const.tile([P, n_tiles], f32, name="sb_tmp")
            sb_din = const.tile([P, n_tiles], f32, name="sb_din")
            nc.scalar.sqrt(out=sb_tmp[:], in_=sb_indeg[:])
            nc.vector.reciprocal(out=sb_din[:], in_=sb_tmp[:])
            sb_tmp2 = const.tile([P, n_tiles], f32, name="sb_tmp2")
            sb_dout = const.tile([P, n_tiles], f32, name="sb_dout")
            nc.scalar.sqrt(out=sb_tmp2[:], in_=sb_outdeg[:])
            nc.vector.reciprocal(out=sb_dout[:], in_=sb_tmp2[:])

            # s0 = feat * dout -> shard [SH, 1] -> AllGather -> expand
            s0st = const.tile([P, n_tiles], f32, name="s0st")
            nc.vector.tensor_tensor(out=s0st[:], in0=sb_feat[:],
                                    in1=sb_dout[:], op=Alu.mult)

            def share_table(sstage, F, l):
                """sstage [P, n_tiles*F] -> AllGather -> expanded T_pad."""
                sfx = "0" if l == 0 else ""
                s_in = dramTp.tile([SH, F], f32, name=f"s_in{l}",
                                   tag=f"s_in{sfx}")
                nc.sync.dma_start(
                    out=s_in[:, :].rearrange("(t p) f -> p t f", p=P),
                    in_=sstage[:, :].rearrange("p (t f) -> p t f", t=n_tiles))
                T_c = dramTp.tile([TBL, F], f32, name=f"Tc{l}",
                                  tag=f"T_c{sfx}", addr_space="Shared")
                nc.gpsimd.collective_compute("AllGather", Alu.bypass, rg,
                                             ins=[s_in[:]], outs=[T_c[:]])
                T_pad = dramTp.tile([TBL, RW], f32, name=f"Tp{l}",
                                    tag="T_pad")
                # expand 8 row-chunks of 12544 via SBUF
                for k in range(NC):
                    cbuf = cbufp.tile([P, n_tiles * F], f32, name=f"cb{l}_{k}",
                                      tag=f"cbuf{sfx}")
                    nc.sync.dma_start(
                        out=cbuf[:].rearrange("p (i f) -> p i f", i=n_tiles),
                        in_=T_c[k * SH:(k + 1) * SH, :].rearrange(
                            "(p i) f -> p i f", p=P))
                    ebuf = ebufp.tile([P, n_tiles * RW], f32, name=f"eb{l}_{k}",
                                      tag="ebuf")
                    nc.vector.tensor_copy(
                        out=ebuf[:].rearrange("p (i f) -> p i f",
                                              i=n_tiles)[:, :, 0:F],
                        in_=cbuf[:].rearrange("p (i f) -> p i f", i=n_tiles))
                    nc.sync.dma_start(
                        out=T_pad[k * SH:(k + 1) * SH, :].rearrange(
                            "(p i) f -> p i f", p=P),
                        in_=ebuf[:].rearrange("p (i f) -> p i f", i=n_tiles))
                return T_pad

            T_pad = share_table(s0st, 1, 0)

            # ---- layers ----
            s_prev = s0st
            for l in range(1, L + 1):
                F = 1 if l == 1 else D
                hstage = stagep.tile([P, n_tiles * D], f32, name=f"hstage{l}",
                                     tag="hstage")
                aggb = []
                for b in range(NB):
                    a = aggp.tile([P, n_tiles * F], f32, name=f"agg{l}_{b}",
                                  tag=f"agg{b}")
                    nc.vector.memset(a[:], 0.0)
                    aggb.append(a)

                # per-bank idx loads + gathers + reduces
                sb_idxb = {}
                for b in range(NB):
                    ib = idxp.tile([P, REGMAX * 8], i16, name=f"idx{l}_{b}",
                                   tag="idxb")
                    nc.sync.dma_start(
                        out=ib[:, 0:int(REG[b]) * 8],
                        in_=d_idx[:, int(BOFF[b]) * 8:int(BOFF[b + 1]) * 8])
                    sb_idxb[b] = ib

                stream_of_chunk = {}
                for ci, (b, t0, t1, col0, cols) in enumerate(chunks):
                    stream = streamp.tile([P, CC * RW], f32,
                                          name=f"st{l}_{ci}", tag="stream")
                    stream_of_chunk[ci] = stream
                    nc.gpsimd.dma_gather(
                        stream[:, 0:cols * RW].rearrange(
                            "p (c f) -> p c f", f=RW),
                        T_pad[b * BR:(b + 1) * BR, :],
                        sb_idxb[b][:, col0 * 8:(col0 + cols) * 8],
                        cols * P, cols * P, RW,
                        single_packet=False, queue_num=b)
                for (ci, colc, jt0, T_run, S_run) in jobs:
                    b = chunks[ci][0]
                    stream = stream_of_chunk[ci]
                    in_ap = stream[:, colc * RW:(colc + T_run * S_run) * RW]
                    in_ap = in_ap.rearrange("p (t j f) -> p t f j",
                                            t=T_run, j=S_run, f=RW)[:, :, 0:F, :]
                    out_ap = aggb[b][:, jt0 * F:(jt0 + T_run) * F].rearrange(
                        "p (t f) -> p t f", t=T_run)
                    nc.vector.tensor_reduce(out=out_ap, in_=in_ap,
                                            axis=Axis.X, op=Alu.add)

                # agg = sum of banks + self-loop term
                nc.vector.tensor_tensor(out=aggb[0][:], in0=aggb[0][:],
                                        in1=aggb[1][:], op=Alu.add)
                nc.vector.tensor_tensor(out=aggb[2][:], in0=aggb[2][:],
                                        in1=aggb[3][:], op=Alu.add)
                nc.vector.tensor_tensor(out=aggb[0][:], in0=aggb[0][:],
                                        in1=aggb[2][:], op=Alu.add)
                nc.vector.tensor_tensor(out=aggb[0][:], in0=aggb[0][:],
                                        in1=s_prev[:], op=Alu.add)
                agg = aggb[0]

                # W multiply
                if l == 1:
                    for t in range(n_tiles):
                        nc.vector.tensor_scalar(
                            out=hstage[:, t * D:(t + 1) * D],
                            in0=sb_wstart[:, :],
                            scalar1=agg[:, t:t + 1], scalar2=None,
                            op0=Alu.mult)
                    h3 = hstage[:].rearrange("p (t f) -> p t f", t=n_tiles)
                    nc.vector.tensor_tensor(
                        out=h3, in0=h3,
                        in1=sb_din[:].unsqueeze(2).to_broadcast([P, n_tiles, D]),
                        op=Alu.mult)
                else:
                    for (g0, gt) in plan["groups"]:
                        tp = ptp.tile([gt * D, P], f32, name=f"tp{l}_{g0}",
                                      tag="tp")
                        nc.tensor.transpose(out=tp[:],
                                            in_=agg[:, g0 * D:(g0 + gt) * D],
                                            identity=sb_ident[:])
                        aT = atp.tile([gt * D, P], f32, name=f"aT{l}_{g0}",
                                      tag="aT")
                        nc.vector.tensor_copy(out=aT[:], in_=tp[:])
                        hp = php.tile([P, gt * D], f32, name=f"hp{l}_{g0}",
                                      tag="hp")
                        wb = sb_wblk[gt]
                        base = (l - 2) * gt * D
                        nc.tensor.matmul(out=hp[:], lhsT=aT[:],
                                         rhs=wb[:, base:base + gt * D],
                                         start=True, stop=True)
                        nc.vector.tensor_tensor(
                            out=hstage[:, g0 * D:(g0 + gt) * D].rearrange(
                                "p (t f) -> p t f", t=gt),
                            in0=hp[:].rearrange("p (t f) -> p t f", t=gt),
                            in1=sb_din[:, g0:g0 + gt].unsqueeze(2)
                                .to_broadcast([P, gt, D]),
                            op=Alu.mult)

                # bias, relu, dout
                h3 = hstage[:].rearrange("p (t f) -> p t f", t=n_tiles)
                nc.vector.tensor_tensor(
                    out=h3, in0=h3,
                    in1=sb_brep[:, (l - 1) * D:l * D].unsqueeze(1)
                        .to_broadcast([P, n_tiles, D]),
                    op=Alu.add)
                if l < L:
                    nc.scalar.activation(out=hstage[:], in_=hstage[:],
                                         func=Act.Relu)
                    sstage = stagep.tile([P, n_tiles * D], f32,
                                         name=f"sstage{l}", tag="sstage")
                    nc.vector.tensor_tensor(
                        out=sstage[:].rearrange("p (t f) -> p t f", t=n_tiles),
                        in0=h3,
                        in1=sb_dout[:].unsqueeze(2).to_broadcast([P, n_tiles, D]),
                        op=Alu.mult)
                    # pad nodes carry s ~ 1e-15 (dout ~ 0) -- negligible
                    T_pad = share_table(sstage, D, l)
                    s_prev = sstage
                else:
                    nc.sync.dma_start(
                        out=t_out[0:full_t * P, :].rearrange(
                            "(t p) f -> p t f", p=P),
                        in_=hstage[:, 0:full_t * D].rearrange(
                            "p (t f) -> p t f", t=full_t))
                    if rem:
                        nc.sync.dma_start(
                            out=t_out[full_t * P:Np, :],
                            in_=hstage[0:rem, full_t * D:(full_t + 1) * D])

    nc.compile()
    return nc


def make_in_maps(inputs, plan, cfg):
    feat = np.asarray(inputs["feat"], np.float32)
    wstart_rep, brep, wblk = _pack_weights(
        inputs["W_start"], inputs["b_start"], inputs["W_mid"], inputs["b_mid"],
        inputs["W_final"], inputs["b_final"], plan, cfg)
    ident = np.eye(P, dtype=np.float32)
    n_tiles, Np, NC, SH = cfg.n_tiles, cfg.Np, cfg.NC, cfg.SH

    in_maps = []
    for c in range(NC):
        feat_t = np.zeros((P, n_tiles), np.float32)
        fv = np.zeros(SH, np.float32)
        fv[:Np] = feat[plan["perms"][c], 0]
        feat_t[:, :] = fv.reshape(n_tiles, P).T
        m = dict(
            feat_t=feat_t,
            indeg_t=plan["indeg_t"][c],
            outdeg_t=plan["outdeg_t"][c],
            idx16=plan["idx16"][c],
            wstart_rep=wstart_rep,
            brep=brep,
            ident=ident,
        )
        for gt in plan["group_sizes"]:
            m[f"wblk{gt}"] = wblk[gt]
        in_maps.append(m)
    return in_maps


def assemble(results, plan, cfg):
    out = np.zeros((cfg.N, cfg.D), np.float32)
    for c in range(cfg.NC):
        out[plan["perms"][c]] = results[c]["out_h"]
    return out[None]


_LAST = {}


def run(inputs, cfg=CFG, trace=False):
    from concourse import bass_utils
    plan = _plan(inputs["src"], inputs["dst"], cfg)
    nc = build_program(plan, cfg)
    in_maps = make_in_maps(inputs, plan, cfg)
    res = bass_utils.run_bass_kernel_spmd(
        nc, in_maps, core_ids=list(range(cfg.NC)), trace=trace)
    _LAST.update(nc=nc, in_maps=in_maps, cfg=cfg)
    return assemble(res.results, plan, cfg), res


def run_again():
    """Re-execute the last-built program (cached NEFF); returns wall seconds."""
    import time

    from concourse import bass_utils
    t0 = time.time()
    bass_utils.run_bass_kernel_spmd(
        _LAST["nc"], _LAST["in_maps"], core_ids=list(range(_LAST["cfg"].NC)))
    return time.time() - t0


def make_repeat_runner():
    """Build the sharded jitted executable ONCE and keep inputs resident on
    device, so repeat calls measure execution (not retrace/recompile/upload).
    Mirrors bass2jax.run_bass_via_pjrt's multi-core branch."""
    import jax
    import numpy as np
    from jax.experimental.shard_map import shard_map
    from jax.sharding import Mesh, NamedSharding, PartitionSpec

    import concourse.mybir as mybir
    from concourse import bass2jax

    nc = _LAST["nc"]
    in_maps = _LAST["in_maps"]
    n_cores = _LAST["cfg"].NC
    bass2jax.install_neuronx_cc_hook()
    partition_name = (nc.partition_id_tensor.name
                      if nc.partition_id_tensor else None)

    in_names, out_names, out_avals = [], [], []
    for alloc in nc.m.functions[0].allocations:
        if not isinstance(alloc, mybir.MemoryLocationSet):
            continue
        name = alloc.memorylocations[0].name
        if alloc.kind == "ExternalInput":
            if name != partition_name:
                in_names.append(name)
        elif alloc.kind == "ExternalOutput":
            out_names.append(name)
            out_avals.append(jax.core.ShapedArray(
                tuple(alloc.tensor_shape), mybir.dt.np(alloc.dtype)))
    n_params = len(in_names)
    all_in = list(in_names) + list(out_names)
    if partition_name is not None:
        all_in.append(partition_name)

    def _body(*args):
        operands = list(args)
        if partition_name is not None:
            operands.append(bass2jax.partition_id_tensor())
        outs = bass2jax._bass_exec_p.bind(
            *operands,
            out_avals=tuple(out_avals),
            in_names=tuple(all_in),
            out_names=tuple(out_names),
            lowering_input_output_aliases=(),
            sim_require_finite=True,
            sim_require_nnan=True,
            nc=nc,
        )
        return tuple(outs)

    devices = jax.devices()[:n_cores]
    mesh = Mesh(np.asarray(devices), ("core",))
    n_outs = len(out_names)
    in_specs = (PartitionSpec("core"),) * (n_params + n_outs)
    out_specs = (PartitionSpec("core"),) * n_outs
    sharded = jax.jit(
        shard_map(_body, mesh=mesh, in_specs=in_specs, out_specs=out_specs,
                  check_rep=False),
        keep_unused=True,
    )
    sh = NamedSharding(mesh, PartitionSpec("core"))
    concat_in = [
        jax.device_put(np.concatenate(
            [np.asarray(in_maps[c][nm]) for c in range(n_cores)], axis=0), sh)
        for nm in in_names
    ]
    concat_zeros = [
        jax.device_put(np.zeros((n_cores * a.shape[0], *a.shape[1:]), a.dtype),
                       sh)
        for a in out_avals
    ]
    out = sharded(*concat_in, *concat_zeros)  # warm: compile + first exec
    jax.block_until_ready(out)

    def repeat_once():
        import time
        t0 = time.time()
        o = sharded(*concat_in, *concat_zeros)
        jax.block_until_ready(o)
        return time.time() - t0, o

    return repeat_once, out_names, out_avals


def kernel(**inputs):
    out, _ = run(inputs)
    return out.astype(np.float32)


# revision 17
# speedup vs baseline: 1.0971x; 1.0971x over previous
"""Trainium2 Bass kernel for nn_BaseGCN (20-layer GraphConv, norm='both').

Distribution (graph/data parallel per the sharding hint): nodes are sharded
contiguously across 8 NeuronCores (12500 each, padded to 12544 = 98 tiles of
128), sorted by in-degree so each 128-node tile has near-uniform in-edge
count.  Per layer each core computes s = dout*h on its shard, AllGathers the
8 shards into a compact DRAM table, expands it to 256B-strided rows, then
fetches the source rows of its in-edges with batched `dma_gather`
instructions (thousands of rows per instruction).  The int16 gather index
forces a 4-way bank split of the 100352-row table; edge slots are laid out
per (bank, tile) so strided DVE reduces segment-sum each bank, the four
bank partials are added, W is applied via PE transpose + block-diagonal
matmul, then din / bias / relu.

Graph *structure* (shard assignment, sort order, slot layout, gather
indices, integer degrees) is computed on the host, like CSR construction in
any GNN framework.  All floating-point math runs on device.
"""

import numpy as np

P = 128   # SBUF partitions
RW = 64   # f32 per table row (256B, dma_gather granularity)
CC = 96   # max slot columns per dma_gather chunk


class _Cfg:
    def __init__(self, N, E, D, n_mid, n_cores):
        self.N = N
        self.E = E
        self.D = D
        self.n_mid = n_mid
        self.L = n_mid + 2
        self.NC = n_cores
        self.Np = N // n_cores          # 12500
        self.n_tiles = -(-self.Np // P)  # 98
        self.SH = self.n_tiles * P       # 12544 padded shard rows
        self.full_t = self.Np // P       # 97
        self.rem = self.Np - self.full_t * P  # 84
        self.NB = 4                      # index banks (int16 limit)
        self.BR = 2 * self.SH            # rows per bank = 25088


CFG = _Cfg(N=100000, E=3200000, D=20, n_mid=18, n_cores=8)


# ---------------------------------------------------------------------------
# Host-side planning (graph structure only)
# ---------------------------------------------------------------------------

def _plan(src, dst, cfg):
    N, NC, Np, n_tiles, SH, NB = (cfg.N, cfg.NC, cfg.Np, cfg.n_tiles, cfg.SH,
                                  cfg.NB)
    src = np.asarray(src, dtype=np.int64)
    dst = np.asarray(dst, dtype=np.int64)

    out_deg = np.bincount(src, minlength=N) + 1  # + self loop
    in_deg = np.bincount(dst, minlength=N) + 1

    core_of = np.arange(N) // Np
    # per-node per-source-bank in-edge counts; bank of a source node is its
    # core pair (gidx // (2*SH) == core // 2, independent of within-core pos)
    bank_of = core_of // 2
    cnb = np.zeros((N, NB), np.int32)
    np.add.at(cnb, (dst, bank_of[src]), 1)
    pos = np.empty(N, np.int64)
    perms = np.zeros((NC, Np), np.int64)
    for c in range(NC):
        nodes = np.arange(c * Np, (c + 1) * Np)
        # group nodes by dominant source bank, then by max / 2nd-max per-bank
        # count: minimizes per-tile per-bank slot maxima (34% fewer gather
        # rows than total-degree sort)
        cb = cnb[nodes]
        sc = np.sort(cb, axis=1)
        order = np.lexsort((sc[:, 2], sc[:, 3], cb.argmax(axis=1)))
        perms[c] = nodes[order]
        pos[nodes[order]] = np.arange(Np)
    gidx = core_of * SH + pos           # row in the expanded table
    bank = gidx // cfg.BR               # 4 banks of 25088 rows

    # per (core, tile, partition, bank) in-edge counts (self-loops local)
    cnt = np.zeros((NC, n_tiles, P, NB), np.int32)
    dp = pos[dst]
    np.add.at(cnt, (core_of[dst], dp // P, dp % P, bank[src]), 1)
    S = cnt.max(axis=2).max(axis=0)     # [n_tiles, NB] shared across cores

    off = np.zeros((NB, n_tiles + 1), np.int64)   # column offsets per bank
    for b in range(NB):
        off[b, 1:] = np.cumsum(S[:, b])
    REG = off[:, -1]                    # region sizes
    BOFF = np.concatenate([[0], np.cumsum(REG)])
    BTOT = int(BOFF[-1])

    # chunks: per bank, tile runs with <= CC columns
    chunks = []   # (bank, t0, t1, col0_rel, cols)
    for b in range(NB):
        t0 = 0
        while t0 < n_tiles:
            while t0 < n_tiles and S[t0, b] == 0:
                t0 += 1
            if t0 >= n_tiles:
                break
            t1 = t0 + 1
            while t1 < n_tiles and off[b, t1 + 1] - off[b, t0] <= CC:
                t1 += 1
            chunks.append((b, t0, t1, int(off[b, t0]),
                           int(off[b, t1] - off[b, t0])))
            t0 = t1

    # reduce jobs: per chunk, runs of equal S
    jobs = []   # (chunk_id, col_in_chunk, t0, T, S)
    for ci, (b, t0, t1, col0, _) in enumerate(chunks):
        t = t0
        while t < t1:
            te = t + 1
            while te < t1 and S[te, b] == S[t, b]:
                te += 1
            jobs.append((ci, int(off[b, t] - col0), int(t), int(te - t),
                         int(S[t, b])))
            t = te

    # gather indices: per core [P, BTOT] local bank row ids
    ZPAD = {b: 2 * b * SH + Np - cfg.BR * b for b in range(NB)}  # pad row
    idx = np.zeros((NC, P, BTOT), np.int32)
    for b in range(NB):
        idx[:, :, BOFF[b]:BOFF[b + 1]] = ZPAD[b]
    ekey = gidx[dst]
    eorder = np.argsort(ekey, kind="stable")
    s_sorted = src[eorder]
    d_sorted = dst[eorder]
    e_core = core_of[d_sorted]
    core_starts = np.searchsorted(e_core, np.arange(NC + 1))
    for c in range(NC):
        lo, hi = core_starts[c], core_starts[c + 1]
        ss = s_sorted[lo:hi]
        dpc = pos[d_sorted[lo:hi]]
        sb = bank[ss]
        # rank of each edge within its (dst-node, bank) group
        key = dpc * NB + sb
        korder = np.argsort(key, kind="stable")
        ksorted = key[korder]
        grp_cnt = np.bincount(ksorted, minlength=Np * NB)
        starts = np.concatenate([[0], np.cumsum(grp_cnt)[:-1]])
        rank = np.arange(len(ss)) - starts[ksorted]
        cols = (BOFF[sb[korder]] + off[sb[korder], dpc[korder] // P]
                + rank)
        lidx = gidx[ss[korder]] - sb[korder] * cfg.BR
        idx[c, dpc[korder] % P, cols] = lidx.astype(np.int32)

    # int16 wrapped layout: flat i = col*128 + p -> (p%16, col*8 + p//16)
    pp = np.arange(P)
    idx16 = np.zeros((NC, 16, BTOT * 8), np.int16)
    for c in range(NC):
        a = np.zeros((16, BTOT * 8), np.int16)
        a[(pp % 16)[:, None],
          np.add.outer(pp // 16, np.arange(BTOT) * 8)] = idx[c].astype(np.int16)
        idx16[c] = a

    def tiles_of(vec_by_core):
        out = np.zeros((NC, P, n_tiles), np.float32)
        for c in range(NC):
            v = np.zeros(SH, np.float32)
            v[:Np] = vec_by_core[c]
            out[c] = v.reshape(n_tiles, P).T
        return out

    indeg_t = tiles_of([np.maximum(in_deg[perms[c]], 1) for c in range(NC)])
    outdeg_t = tiles_of([np.maximum(out_deg[perms[c]], 1) for c in range(NC)])
    indeg_t[indeg_t == 0] = 1.0
    # pad nodes: dout = 1/sqrt(1e30) ~ 0 so their table rows stay ~0
    outdeg_t[outdeg_t == 0] = 1e30

    groups = [(g0, min(4, n_tiles - g0)) for g0 in range(0, n_tiles, 4)]
    group_sizes = sorted(set(gt for _, gt in groups))

    return dict(perms=perms, S=S, off=off, REG=REG, BOFF=BOFF, BTOT=BTOT,
                chunks=chunks, jobs=jobs, idx16=idx16, indeg_t=indeg_t,
                outdeg_t=outdeg_t, groups=groups, group_sizes=group_sizes)


def _pack_weights(W_start, b_start, W_mid, b_mid, W_final, b_final, plan, cfg):
    D, L = cfg.D, cfg.L
    Ws = [np.asarray(W_start, np.float32)]
    bs = [np.asarray(b_start, np.float32)]
    for i in range(cfg.n_mid):
        Ws.append(np.asarray(W_mid[i], np.float32))
        bs.append(np.asarray(b_mid[i], np.float32))
    Ws.append(np.asarray(W_final, np.float32))
    bs.append(np.asarray(b_final, np.float32))

    wstart_rep = np.tile(Ws[0][0][None, :], (P, 1)).astype(np.float32)
    brep = np.zeros((P, L * D), np.float32)
    for l in range(L):
        brep[:, l * D:(l + 1) * D] = bs[l][None, :]

    wblk = {}
    for gt in plan["group_sizes"]:
        M = np.zeros((gt * D, (L - 1) * gt * D), np.float32)
        for l in range(2, L + 1):
            W = Ws[l - 1]
            base = (l - 2) * gt * D
            for k in range(gt):
                M[k * D:(k + 1) * D, base + k * D:base + (k + 1) * D] = W
        wblk[gt] = M
    return wstart_rep, brep, wblk


# ---------------------------------------------------------------------------
# Device program
# ---------------------------------------------------------------------------

def build_program(plan, cfg):
    import concourse.bacc as bacc
    import concourse.mybir as mybir
    import concourse.tile as tile

    f32 = mybir.dt.float32
    i16 = mybir.dt.int16
    Alu = mybir.AluOpType
    Axis = mybir.AxisListType
    Act = mybir.ActivationFunctionType

    D, L, NC, Np = cfg.D, cfg.L, cfg.NC, cfg.Np
    n_tiles, full_t, rem, SH, NB, BR = (cfg.n_tiles, cfg.full_t, cfg.rem,
                                        cfg.SH, cfg.NB, cfg.BR)
    BTOT = plan["BTOT"]
    REG, BOFF = plan["REG"], plan["BOFF"]
    chunks, jobs = plan["chunks"], plan["jobs"]
    REGMAX = int(max(REG))
    TBL = NC * SH                        # 100352

    nc = bacc.Bacc("TRN2", target_bir_lowering=False, debug=False,
                   enable_asserts=False, num_devices=NC, num_swdge_queues=4)

    t_feat = nc.dram_tensor("feat_t", [P, n_tiles], f32, kind="ExternalInput").ap()
    t_indeg = nc.dram_tensor("indeg_t", [P, n_tiles], f32, kind="ExternalInput").ap()
    t_outdeg = nc.dram_tensor("outdeg_t", [P, n_tiles], f32, kind="ExternalInput").ap()
    t_idx16 = nc.dram_tensor("idx16", [16, BTOT * 8], i16, kind="ExternalInput").ap()
    t_wstart = nc.dram_tensor("wstart_rep", [P, D], f32, kind="ExternalInput").ap()
    t_brep = nc.dram_tensor("brep", [P, L * D], f32, kind="ExternalInput").ap()
    t_wblk = {}
    for gt in plan["group_sizes"]:
        t_wblk[gt] = nc.dram_tensor(f"wblk{gt}", [gt * D, (L - 1) * gt * D],
                                    f32, kind="ExternalInput").ap()
    t_ident = nc.dram_tensor("ident", [P, P], f32, kind="ExternalInput").ap()
    t_out = nc.dram_tensor("out_h", [Np, D], f32, kind="ExternalOutput").ap()

    rg = [list(range(NC))]

    with tile.TileContext(nc) as tc:
        with (
            tc.tile_pool(name="const", bufs=1) as const,
            tc.tile_pool(name="idxp", bufs=1) as idxp,
            tc.tile_pool(name="stream", bufs=2) as streamp,
            tc.tile_pool(name="aggp", bufs=1) as aggp,
            tc.tile_pool(name="stage", bufs=2) as stagep,
            tc.tile_pool(name="cbufp", bufs=2) as cbufp,
            tc.tile_pool(name="ebufp", bufs=1) as ebufp,
            tc.tile_pool(name="atp", bufs=2) as atp,
            tc.tile_pool(name="ptp", bufs=2, space="PSUM") as ptp,
            tc.tile_pool(name="php", bufs=2, space="PSUM") as php,
            tc.tile_pool(name="dram", bufs=1, space="DRAM") as dramp,
            tc.tile_pool(name="dramT", bufs=2, space="DRAM") as dramTp,
        ):
            # ---- prologue ----
            sb_feat = const.tile([P, n_tiles], f32, name="sb_feat")
            nc.sync.dma_start(out=sb_feat[:], in_=t_feat[:])
            sb_indeg = const.tile([P, n_tiles], f32, name="sb_indeg")
            nc.sync.dma_start(out=sb_indeg[:], in_=t_indeg[:])
            sb_outdeg = const.tile([P, n_tiles], f32, name="sb_outdeg")
            nc.sync.dma_start(out=sb_outdeg[:], in_=t_outdeg[:])
            sb_wstart = const.tile([P, D], f32, name="sb_wstart")
            nc.sync.dma_start(out=sb_wstart[:], in_=t_wstart[:])
            sb_brep = const.tile([P, L * D], f32, name="sb_brep")
            nc.sync.dma_start(out=sb_brep[:], in_=t_brep[:])
            sb_wblk = {}
            for gt in plan["group_sizes"]:
                sb_wblk[gt] = const.tile([gt * D, (L - 1) * gt * D], f32,
                                         name=f"sb_wblk{gt}")
                nc.sync.dma_start(out=sb_wblk[gt][:], in_=t_wblk[gt][:])
            sb_ident = const.tile([P, P], f32, name="sb_ident")
            nc.sync.dma_start(out=sb_ident[:], in_=t_ident[:])

            # replicate the 16-row idx block to 128 partitions in DRAM
            d_idx = dramp.tile([P, BTOT * 8], i16, name="d_idx", tag="d_idx")
            for g in range(8):
                nc.sync.dma_start(out=d_idx[16 * g:16 * (g + 1), :],
                                  in_=t_idx16[:, :])

            sb_tmp = const.tile([P, n_tiles], f32, name="sb_tmp")
            sb_din = const.tile([P, n_tiles], f32, name="sb_din")
            nc.scalar.sqrt(out=sb_tmp[:], in_=sb_indeg[:])
            nc.vector.reciprocal(out=sb_din[:], in_=sb_tmp[:])
            sb_tmp2 = const.tile([P, n_tiles], f32, name="sb_tmp2")
            sb_dout = const.tile([P, n_tiles], f32, name="sb_dout")
            nc.scalar.sqrt(out=sb_tmp2[:], in_=sb_outdeg[:])
            nc.vector.reciprocal(out=sb_dout[:], in_=sb_tmp2[:])

            # s0 = feat * dout -> shard [SH, 1] -> AllGather -> expand
            s0st = const.tile([P, n_tiles], f32, name="s0st")
            nc.vector.tensor_tensor(out=s0st[:], in0=sb_feat[:],
                                    in1=sb_dout[:], op=Alu.mult)

            def share_table(sstage, F, l):
                """sstage [P, n_tiles*F] -> AllGather -> expanded T_pad."""
                sfx = "0" if l == 0 else ""
                s_in = dramTp.tile([SH, F], f32, name=f"s_in{l}",
                                   tag=f"s_in{sfx}")
                nc.sync.dma_start(
                    out=s_in[:, :].rearrange("(t p) f -> p t f", p=P),
                    in_=sstage[:, :].rearrange("p (t f) -> p t f", t=n_tiles))
                T_c = dramTp.tile([TBL, F], f32, name=f"Tc{l}",
                                  tag=f"T_c{sfx}", addr_space="Shared")
                nc.gpsimd.collective_compute("AllGather", Alu.bypass, rg,
                                             ins=[s_in[:]], outs=[T_c[:]])
                T_pad = dramTp.tile([TBL, RW], f32, name=f"Tp{l}",
                                    tag="T_pad")
                # expand 8 row-chunks of 12544 via SBUF
                for k in range(NC):
                    cbuf = cbufp.tile([P, n_tiles * F], f32, name=f"cb{l}_{k}",
                                      tag=f"cbuf{sfx}")
                    nc.sync.dma_start(
                        out=cbuf[:].rearrange("p (i f) -> p i f", i=n_tiles),
                        in_=T_c[k * SH:(k + 1) * SH, :].rearrange(
                            "(p i) f -> p i f", p=P))
                    ebuf = ebufp.tile([P, n_tiles * RW], f32, name=f"eb{l}_{k}",
                                      tag="ebuf")
                    nc.vector.tensor_copy(
                        out=ebuf[:].rearrange("p (i f) -> p i f",
                                              i=n_tiles)[:, :, 0:F],
                        in_=cbuf[:].rearrange("p (i f) -> p i f", i=n_tiles))
                    nc.sync.dma_start(
                        out=T_pad[k * SH:(k + 1) * SH, :].rearrange(
                            "(p i) f -> p i f", p=P),
                        in_=ebuf[:].rearrange("p (i f) -> p i f", i=n_tiles))
                return T_pad

            T_pad = share_table(s0st, 1, 0)

            # ---- layers ----
            s_prev = s0st
            for l in range(1, L + 1):
                F = 1 if l == 1 else D
                hstage = stagep.tile([P, n_tiles * D], f32, name=f"hstage{l}",
                                     tag="hstage")
                aggb = []
                for b in range(NB):
                    a = aggp.tile([P, n_tiles * F], f32, name=f"agg{l}_{b}",
                                  tag=f"agg{b}")
                    nc.vector.memset(a[:], 0.0)
                    aggb.append(a)

                # per-bank idx loads + gathers + reduces
                sb_idxb = {}
                for b in range(NB):
                    ib = idxp.tile([P, REGMAX * 8], i16, name=f"idx{l}_{b}",
                                   tag="idxb")
                    nc.sync.dma_start(
                        out=ib[:, 0:int(REG[b]) * 8],
                        in_=d_idx[:, int(BOFF[b]) * 8:int(BOFF[b + 1]) * 8])
                    sb_idxb[b] = ib

                stream_of_chunk = {}
                for ci, (b, t0, t1, col0, cols) in enumerate(chunks):
                    stream = streamp.tile([P, CC * RW], f32,
                                          name=f"st{l}_{ci}", tag="stream")
                    stream_of_chunk[ci] = stream
                    nc.gpsimd.dma_gather(
                        stream[:, 0:cols * RW].rearrange(
                            "p (c f) -> p c f", f=RW),
                        T_pad[b * BR:(b + 1) * BR, :],
                        sb_idxb[b][:, col0 * 8:(col0 + cols) * 8],
                        cols * P, cols * P, RW,
                        single_packet=False, queue_num=b)
                for (ci, colc, jt0, T_run, S_run) in jobs:
                    b = chunks[ci][0]
                    stream = stream_of_chunk[ci]
                    in_ap = stream[:, colc * RW:(colc + T_run * S_run) * RW]
                    in_ap = in_ap.rearrange("p (t j f) -> p t f j",
                                            t=T_run, j=S_run, f=RW)[:, :, 0:F, :]
                    out_ap = aggb[b][:, jt0 * F:(jt0 + T_run) * F].rearrange(
                        "p (t f) -> p t f", t=T_run)
                    nc.vector.tensor_reduce(out=out_ap, in_=in_ap,
                                            axis=Axis.X, op=Alu.add)

                # agg = sum of banks + self-loop term
                nc.vector.tensor_tensor(out=aggb[0][:], in0=aggb[0][:],
                                        in1=aggb[1][:], op=Alu.add)
                nc.vector.tensor_tensor(out=aggb[2][:], in0=aggb[2][:],
                                        in1=aggb[3][:], op=Alu.add)
                nc.vector.tensor_tensor(out=aggb[0][:], in0=aggb[0][:],
                                        in1=aggb[2][:], op=Alu.add)
                nc.vector.tensor_tensor(out=aggb[0][:], in0=aggb[0][:],
                                        in1=s_prev[:], op=Alu.add)
                agg = aggb[0]

                # W multiply
                if l == 1:
                    for t in range(n_tiles):
                        nc.vector.tensor_scalar(
                            out=hstage[:, t * D:(t + 1) * D],
                            in0=sb_wstart[:, :],
                            scalar1=agg[:, t:t + 1], scalar2=None,
                            op0=Alu.mult)
                    h3 = hstage[:].rearrange("p (t f) -> p t f", t=n_tiles)
                    nc.vector.tensor_tensor(
                        out=h3, in0=h3,
                        in1=sb_din[:].unsqueeze(2).to_broadcast([P, n_tiles, D]),
                        op=Alu.mult)
                else:
                    for (g0, gt) in plan["groups"]:
                        tp = ptp.tile([gt * D, P], f32, name=f"tp{l}_{g0}",
                                      tag="tp")
                        nc.tensor.transpose(out=tp[:],
                                            in_=agg[:, g0 * D:(g0 + gt) * D],
                                            identity=sb_ident[:])
                        aT = atp.tile([gt * D, P], f32, name=f"aT{l}_{g0}",
                                      tag="aT")
                        nc.vector.tensor_copy(out=aT[:], in_=tp[:])
                        hp = php.tile([P, gt * D], f32, name=f"hp{l}_{g0}",
                                      tag="hp")
                        wb = sb_wblk[gt]
                        base = (l - 2) * gt * D
                        nc.tensor.matmul(out=hp[:], lhsT=aT[:],
                                         rhs=wb[:, base:base + gt * D],
                                         start=True, stop=True)
                        nc.vector.tensor_tensor(
                            out=hstage[:, g0 * D:(g0 + gt) * D].rearrange(
                                "p (t f) -> p t f", t=gt),
                            in0=hp[:].rearrange("p (t f) -> p t f", t=gt),
                            in1=sb_din[:, g0:g0 + gt].unsqueeze(2)
                                .to_broadcast([P, gt, D]),
                            op=Alu.mult)

                # bias, relu, dout
                h3 = hstage[:].rearrange("p (t f) -> p t f", t=n_tiles)
                nc.vector.tensor_tensor(
                    out=h3, in0=h3,
                    in1=sb_brep[:, (l - 1) * D:l * D].unsqueeze(1)
                        .to_broadcast([P, n_tiles, D]),
                    op=Alu.add)
                if l < L:
                    nc.scalar.activation(out=hstage[:], in_=hstage[:],
                                         func=Act.Relu)
                    sstage = stagep.tile([P, n_tiles * D], f32,
                                         name=f"sstage{l}", tag="sstage")
                    nc.vector.tensor_tensor(
                        out=sstage[:].rearrange("p (t f) -> p t f", t=n_tiles),
                        in0=h3,
                        in1=sb_dout[:].unsqueeze(2).to_broadcast([P, n_tiles, D]),
                        op=Alu.mult)
                    # pad nodes carry s ~ 1e-15 (dout ~ 0) -- negligible
                    T_pad = share_table(sstage, D, l)
                    s_prev = sstage
                else:
                    nc.sync.dma_start(
                        out=t_out[0:full_t * P, :].rearrange(
                            "(t p) f -> p t f", p=P),
                        in_=hstage[:, 0:full_t * D].rearrange(
                            "p (t f) -> p t f", t=full_t))
                    if rem:
                        nc.sync.dma_start(
                            out=t_out[full_t * P:Np, :],
                            in_=hstage[0:rem, full_t * D:(full_t + 1) * D])

    nc.compile()
    return nc


def make_in_maps(inputs, plan, cfg):
    feat = np.asarray(inputs["feat"], np.float32)
    wstart_rep, brep, wblk = _pack_weights(
        inputs["W_start"], inputs["b_start"], inputs["W_mid"], inputs["b_mid"],
        inputs["W_final"], inputs["b_final"], plan, cfg)
    ident = np.eye(P, dtype=np.float32)
    n_tiles, Np, NC, SH = cfg.n_tiles, cfg.Np, cfg.NC, cfg.SH

    in_maps = []
    for c in range(NC):
        feat_t = np.zeros((P, n_tiles), np.float32)
        fv = np.zeros(SH, np.float32)
        fv[:Np] = feat[plan["perms"][c], 0]
        feat_t[:, :] = fv.reshape(n_tiles, P).T
        m = dict(
            feat_t=feat_t,
            indeg_t=plan["indeg_t"][c],
            outdeg_t=plan["outdeg_t"][c],
            idx16=plan["idx16"][c],
            wstart_rep=wstart_rep,
            brep=brep,
            ident=ident,
        )
        for gt in plan["group_sizes"]:
            m[f"wblk{gt}"] = wblk[gt]
        in_maps.append(m)
    return in_maps


def assemble(results, plan, cfg):
    out = np.zeros((cfg.N, cfg.D), np.float32)
    for c in range(cfg.NC):
        out[plan["perms"][c]] = results[c]["out_h"]
    return out[None]


_LAST = {}


def run(inputs, cfg=CFG, trace=False):
    from concourse import bass_utils
    plan = _plan(inputs["src"], inputs["dst"], cfg)
    nc = build_program(plan, cfg)
    in_maps = make_in_maps(inputs, plan, cfg)
    res = bass_utils.run_bass_kernel_spmd(
        nc, in_maps, core_ids=list(range(cfg.NC)), trace=trace)
    _LAST.update(nc=nc, in_maps=in_maps, cfg=cfg)
    return assemble(res.results, plan, cfg), res


def run_again():
    """Re-execute the last-built program (cached NEFF); returns wall seconds."""
    import time

    from concourse import bass_utils
    t0 = time.time()
    bass_utils.run_bass_kernel_spmd(
        _LAST["nc"], _LAST["in_maps"], core_ids=list(range(_LAST["cfg"].NC)))
    return time.time() - t0


def make_repeat_runner():
    """Build the sharded jitted executable ONCE and keep inputs resident on
    device, so repeat calls measure execution (not retrace/recompile/upload).
    Mirrors bass2jax.run_bass_via_pjrt's multi-core branch."""
    import jax
    import numpy as np
    from jax.experimental.shard_map import shard_map
    from jax.sharding import Mesh, NamedSharding, PartitionSpec

    import concourse.mybir as mybir
    from concourse import bass2jax

    nc = _LAST["nc"]
    in_maps = _LAST["in_maps"]
    n_cores = _LAST["cfg"].NC
    bass2jax.install_neuronx_cc_hook()
    partition_name = (nc.partition_id_tensor.name
                      if nc.partition_id_tensor else None)

    in_names, out_names, out_avals = [], [], []
    for alloc in nc.m.functions[0].allocations:
        if not isinstance(alloc, mybir.MemoryLocationSet):
            continue
        name = alloc.memorylocations[0].name
        if alloc.kind == "ExternalInput":
            if name != partition_name:
                in_names.append(name)
        elif alloc.kind == "ExternalOutput":
            out_names.append(name)
            out_avals.append(jax.core.ShapedArray(
                tuple(alloc.tensor_shape), mybir.dt.np(alloc.dtype)))
    n_params = len(in_names)
    all_in = list(in_names) + list(out_names)
    if partition_name is not None:
        all_in.append(partition_name)

    def _body(*args):
        operands = list(args)
        if partition_name is not None:
            operands.append(bass2jax.partition_id_tensor())
        outs = bass2jax._bass_exec_p.bind(
            *operands,
            out_avals=tuple(out_avals),
            in_names=tuple(all_in),
            out_names=tuple(out_names),
            lowering_input_output_aliases=(),
            sim_require_finite=True,
            sim_require_nnan=True,
            nc=nc,
        )
        return tuple(outs)

    devices = jax.devices()[:n_cores]
    mesh = Mesh(np.asarray(devices), ("core",))
    n_outs = len(out_names)
    in_specs = (PartitionSpec("core"),) * (n_params + n_outs)
    out_specs = (PartitionSpec("core"),) * n_outs
    sharded = jax.jit(
        shard_map(_body, mesh=mesh, in_specs=in_specs, out_specs=out_specs,
                  check_rep=False),
        keep_unused=True,
    )
    sh = NamedSharding(mesh, PartitionSpec("core"))
    concat_in = [
        jax.device_put(np.concatenate(
            [np.asarray(in_maps[c][nm]) for c in range(n_cores)], axis=0), sh)
        for nm in in_names
    ]
    concat_zeros = [
        jax.device_put(np.zeros((n_cores * a.shape[0], *a.shape[1:]), a.dtype),
                       sh)
        for a in out_avals
    ]
    out = sharded(*concat_in, *concat_zeros)  # warm: compile + first exec
    jax.block_until_ready(out)

    def repeat_once():
        import time
        t0 = time.time()
        o = sharded(*concat_in, *concat_zeros)
        jax.block_until_ready(o)
        return time.time() - t0, o

    return repeat_once, out_names, out_avals


def kernel(**inputs):
    out, _ = run(inputs)
    return out.astype(np.float32)
